# revision 43
# baseline (speedup 1.0000x reference)
"""Bass/Trainium2 kernel for nn_MAC_30554397344312 (gnn_message_passing).

Reference computation (B=256 rollout groups, n=64 agents, D=256):
    comm = h @ W_act.T + b_act                      # (B*n, D)
    agg[b,j] = sum_i mask[i,j] * comm[b,i] / (n-1)  # mask = ones - eye
    x   = agg @ W_sum.T + b_sum
    out = relu(x @ W_head.T + b_head)

Everything before the relu is linear, so fold on host:
    Wc = W_head @ W_sum @ W_act          (256x256)
    bc = b_head + b_sum @ W_head.T + b_act @ (W_head @ W_sum).T
    out[b,j] = relu( (A @ H_b)[j] @ Wc.T + bc ),  A = ones-eye (1/63 in Wc)

Layout trick: partition p holds DRAM row pair (2p, 2p+1) of each
256-row chunk ("(n p s) d -> p (n s) d"), so every load/store
descriptor covers 2 KiB contiguous DRAM - half the descriptor count
and DGE time of the naive row-per-partition layout. The aggregation
then needs two mask matrices per output tile (same-parity
blockdiag(ones32)-eye and cross-parity blockdiag(ones32)) accumulated
in PSUM - twice the stage-1 matmuls, which the PE absorbs.

Per core (2048 rows = 8 chunks of 2 tiles), a chunked pipeline:
    load: SWDGE (gpsimd) ring with inline f32->fp16 cast, so no
        compute engine casts h and both HWDGE rings stay free
    stage 1 (PE fp16): Y.T tiles [d, tok] = sum_parity
        matmul(lhsT=H_sub[128t,128d], rhs=A_parity)
    evict Y.T PSUM -> SBUF fp16 on DVE
    stage 3 (PE fp16): out[tok, dout] = Y.T.T @ Wc.T over 2 k-chunks
    relu+descale evict on ACT
    store: 2-tile chunks alternating sync / scalar HWDGE rings
    a dependency-free dummy-matmul burst after the preamble warms the
    PE HAM window so the real matmul stream runs at 2.4 GHz

Sharding: data-parallel over the B axis, 8 cores x 32 groups.
"""

from contextlib import ExitStack

import numpy as np

import concourse.bacc as bacc
import concourse.bass as bass
import concourse.tile as tile
from concourse import mybir
from concourse.bass_utils import run_bass_kernel_spmd

N_AGENTS = 64
B = 256
D = 256
N_CORES = 8
ROWS = B * N_AGENTS            # 16384
ROWS_PER_CORE = ROWS // N_CORES  # 2048
P = 128
N_TILES = ROWS_PER_CORE // P   # 16 token tiles per core
LC = 2                         # tiles per load/store chunk (256 KiB in)
N_LCHUNKS = N_TILES // LC      # 8
BT = 2                         # tiles per agg/main batch
W_SCALE = 16.0  # fp16 weight prescale (power of 2; inverted exactly in relu)
N_WARM = 40  # warm-up matmuls: sized so the burst overlaps the first
             # real matmuls - any PE idle gap before HAM unthrottles
             # poisons the activity window and leaves the stream at 1.2GHz

_cache = {}


def _build(has_bias: bool):
    f32 = mybir.dt.float32
    f16 = mybir.dt.float16
    inv_scale = 1.0 / W_SCALE
    nc = bacc.Bacc("TRN2", target_bir_lowering=False, debug=False,
                   num_devices=N_CORES)

    h = nc.dram_tensor("h", [ROWS_PER_CORE, D], f32, kind="ExternalInput")
    wcT = nc.dram_tensor("wcT", [D, D], f16, kind="ExternalInput")
    ablk = nc.dram_tensor("ablk", [P, 2, P], f16, kind="ExternalInput")
    if has_bias:
        bc = nc.dram_tensor("bc", [1, D], f32, kind="ExternalInput")
    out = nc.dram_tensor("out", [ROWS_PER_CORE, D], f32, kind="ExternalOutput")

    # partition p <-> row pair (2p, 2p+1) per 256-row chunk: 2 KiB descs
    h_ap = h[:, :].rearrange("(n p s) d -> p n s d", p=P, s=2)
    out_ap = out[:, :].rearrange("(n p s) d -> p n s d", p=P, s=2)

    with tile.TileContext(nc) as tc:
        with ExitStack() as ctx:
            const = ctx.enter_context(tc.tile_pool(name="const", bufs=1))
            ytps = ctx.enter_context(
                tc.tile_pool(name="ytps", bufs=4, space="PSUM"))
            outps = ctx.enter_context(
                tc.tile_pool(name="outps", bufs=3, space="PSUM"))
            wps = ctx.enter_context(
                tc.tile_pool(name="warmps", bufs=1, space="PSUM"))

            # h loads: SWDGE ring, f32->fp16 cast inline in the DMA; the
            # last two chunks ride one 4-tile DMA so the final batch's data
            # arrives one queue-boundary sooner
            hc = [const.tile([P, LC, D], f16, tag=f"hc{c}", name=f"hc_{c}")
                  for c in range(N_LCHUNKS - 2)]
            hct = const.tile([P, 2, LC, D], f16, tag="hct", name="hc_tail")
            hc.append(hct[:, 0, :, :])
            hc.append(hct[:, 1, :, :])
            for c in range(N_LCHUNKS - 2):
                nc.gpsimd.dma_start(out=hc[c][:],
                                    in_=h_ap[:, c, :, :])
            nc.gpsimd.dma_start(out=hct[:],
                                in_=h_ap[:, N_LCHUNKS - 2:N_LCHUNKS, :, :])
            # masks on sync ring, weights on scalar ring
            a_t = const.tile([P, 2, P], f16, tag="a", name="a_t")
            nc.sync.dma_start(out=a_t[:], in_=ablk[:, :, :])
            w_t = [const.tile([P, D], f16, tag=f"w{k}", name=f"w_{k}")
                   for k in range(2)]
            for k in range(2):
                nc.scalar.dma_start(out=w_t[k][:],
                                    in_=wcT[k * P:(k + 1) * P, :])
            if has_bias:
                bc_t = const.tile([P, D], f32, tag="bc", name="bc_t")
                bc_bcast = bass.AP(tensor=bc, offset=0, ap=[[0, P], [1, D]])
                nc.gpsimd.dma_start(out=bc_t[:], in_=bc_bcast)

            yt = [const.tile([P, ROWS_PER_CORE], f16, tag=f"yt{k}",
                             name=f"yt_{k}") for k in range(2)]
            och = [const.tile([P, LC, D], f32, tag=f"oc{c}", name=f"oc_{c}")
                   for c in range(N_LCHUNKS)]

            # PE warm-up: dependency-free dummies on a memset tile keep the
            # HAM activity window busy through the load latency
            dw = const.tile([P, P], f16, tag="dw", name="dw")
            nc.vector.memset(dw[:], 0.0)
            dps = wps.tile([P, P], f32, tag="warm", name="warm_ps")
            for _ in range(N_WARM):
                nc.tensor.matmul(dps[:], dw[:], dw[:], start=True, stop=True)

            def agg_batch(b):
                # per output sub-tile: accumulate same-parity (A-eye) and
                # cross-parity (A) sender contributions
                ps = [ytps.tile([P, BT * P], f32, tag="ytps", name="yt_ps")
                      for _ in range(2)]
                for half in range(BT // LC):
                    c = b * (BT // LC) + half
                    for k in range(2):
                        for sub in range(2):        # output parity
                            dst = ps[k][:, (half * LC + sub) * P:
                                        (half * LC + sub + 1) * P]
                            for snd in range(2):    # sender parity
                                lhsT = hc[c][:, snd, k * P:(k + 1) * P]
                                nc.tensor.matmul(
                                    dst, lhsT, a_t[:, sub ^ snd, :],
                                    start=(snd == 0), stop=(snd == 1))
                for k in range(2):
                    nc.vector.tensor_copy(
                        yt[k][:, b * BT * P:(b + 1) * BT * P], ps[k][:])

            deferred = []

            def main_batch(b, split_tail=False):
                for half in range(BT // LC):
                    c = b * (BT // LC) + half           # 2-tile store chunk
                    po = outps.tile([P, LC * D], f32, tag="outps", name="po")
                    for s in range(LC):
                        m = c * LC + s
                        for k in range(2):
                            nc.tensor.matmul(
                                po[:, s * D:(s + 1) * D],
                                yt[k][:, m * P:(m + 1) * P], w_t[k][:],
                                start=(k == 0), stop=(k == 1))
                    dst = och[c][:, :, :]
                    if has_bias:
                        for s in range(LC):
                            d1 = och[c][:, s, :]
                            nc.vector.tensor_scalar(
                                out=d1, in0=po[:, s * D:(s + 1) * D],
                                scalar1=inv_scale, scalar2=None,
                                op0=mybir.AluOpType.mult)
                            nc.vector.tensor_tensor(
                                out=d1, in0=d1, in1=bc_t[:],
                                op=mybir.AluOpType.add)
                            nc.scalar.activation(
                                out=d1, in_=d1,
                                func=mybir.ActivationFunctionType.Relu)
                    elif split_tail:
                        # last chunk: relu halves on DVE & ACT in parallel,
                        # store halves on both rings in parallel
                        nc.vector.tensor_scalar(
                            out=och[c][:, 0, :], in0=po[:, :D],
                            scalar1=inv_scale, scalar2=0.0,
                            op0=mybir.AluOpType.mult,
                            op1=mybir.AluOpType.max)
                        nc.scalar.activation(
                            out=och[c][:, 1, :], in_=po[:, D:],
                            func=mybir.ActivationFunctionType.Relu,
                            scale=inv_scale)
                        nc.sync.dma_start(
                            out=out_ap[:, c, 0, :], in_=och[c][:, 0, :])
                        deferred.append(c)
                        continue
                    else:
                        nc.scalar.activation(
                            out=dst, in_=po[:],
                            func=mybir.ActivationFunctionType.Relu,
                            scale=inv_scale)
                    eng = nc.sync if c % 2 == 0 else nc.scalar
                    eng.dma_start(
                        out=out_ap[:, c, :, :], in_=och[c][:])

            agg_batch(0)
            agg_batch(1)
            for b in range(2, N_LCHUNKS - 1):
                main_batch(b - 2)
                agg_batch(b)
            agg_batch(N_LCHUNKS - 1)
            main_batch(N_LCHUNKS - 3)
            main_batch(N_LCHUNKS - 2, split_tail=True)
            main_batch(N_LCHUNKS - 1, split_tail=True)
            for c in deferred:
                nc.scalar.dma_start(
                    out=out_ap[:, c, 1, :], in_=och[c][:, 1, :])
    nc.finalize()
    return nc


def _fold(W_act, b_act, W_sum, b_sum, W_head, b_head):
    Wa = W_act.astype(np.float64)
    Ws = W_sum.astype(np.float64)
    Wh = W_head.astype(np.float64)
    Wc = Wh @ Ws @ Wa
    bc = (b_head.astype(np.float64)
          + b_sum.astype(np.float64) @ Wh.T
          + b_act.astype(np.float64) @ (Wh @ Ws).T)
    # row-pair layout: partition p <-> tokens (2p, 2p+1); a 128-token
    # sub-tile holds one parity class of a 256-token chunk, so each
    # group of 64 consecutive tokens maps to 32 consecutive partitions.
    ones32 = np.ones((32, 32))
    blk = np.kron(np.eye(4), ones32)        # same-group mask, 128x128
    A_same = (blk - np.eye(P)).astype(np.float16)   # same parity: drop self
    A_cross = blk.astype(np.float16)                # cross parity
    Ablk = np.stack([A_same, A_cross], axis=1)      # [128, 2, 128]
    WcT = (Wc.T / (N_AGENTS - 1) * W_SCALE).astype(np.float16)
    return np.ascontiguousarray(WcT), bc.astype(np.float32), \
        np.ascontiguousarray(Ablk)


def kernel(hidden_state, W_act, b_act, W_sum, b_sum, W_head, b_head,
           _trace=False, _tmpdir=None):
    h = np.ascontiguousarray(np.asarray(hidden_state, dtype=np.float32))
    WcT, bc, Ablk = _fold(np.asarray(W_act), np.asarray(b_act),
                          np.asarray(W_sum), np.asarray(b_sum),
                          np.asarray(W_head), np.asarray(b_head))
    has_bias = bool(np.any(bc))
    if has_bias not in _cache:
        _cache[has_bias] = _build(has_bias)
    nc = _cache[has_bias]

    in_maps = []
    for c in range(N_CORES):
        m = {"h": h[c * ROWS_PER_CORE:(c + 1) * ROWS_PER_CORE],
             "wcT": WcT, "ablk": Ablk}
        if has_bias:
            m["bc"] = bc.reshape(1, D)
        in_maps.append(m)

    res = run_bass_kernel_spmd(
        nc, in_maps, core_ids=list(range(N_CORES)),
        trace=_trace, tmpdir=_tmpdir)
    out = np.concatenate([res.results[c]["out"] for c in range(N_CORES)],
                         axis=0)
    if _trace:
        return out, res
    return out
